# revision 1
# baseline (speedup 1.0000x reference)
"""Trainium2 Bass kernel for the quantized ResNet Bottleneck block.

Strategy
--------
Data parallel over batch: 64 images -> 8 cores x 8 images. Each core runs an
identical Bass program; weights are replicated.

All convs are executed as integer-valued matmuls accumulated in fp32 PSUM
(exact: quantized codes are integers; offsets pass through each conv as
per-output-channel constants folded into the next bias on the host).

Numeric tricks (all exact up to tie-perturbations far below budget):
  * x-quant: two 4x-mode DVE tensor_scalars on the f16 input. Op 1
    computes 254*x + 383 with bf16 output -- in [256, 512) the bf16 ulp
    is 1, so the convert rounds RNE to the integer grid; op 2 clamps to
    [256, 510]. Out-of-binade values land outside the clamp range, so
    this equals clip(rne(254x), -127, 127) + 383 exactly.
  * conv1/conv2 epilogues: ACT computes relu(a*psum + beta) (beta read
    per-partition, no broadcast tiles); a DVE tensor_scalar then does
    (min 127.25) + 384 with bf16 output -- same ulp-1 rounding, and 384
    is even so tie parity matches jnp.round. Codes live in [384, 511]
    (exact in bf16); offsets fold into the next layer's bias via weight
    column sums (x: 383; p2/t3 and the conv2 pad value: 384).

Schedule: x ships from host as f16 (halves input DMA; rel-err impact
~2.6e-3, budget 2e-2) and y returns as bf16 (host upcasts; ~0.2% rel).
x loads are one DMA per image into a per-pair [q, i, k, h] tile (pair 0:
split into half-DMAs + finer quant slices so conv1 starts ~4us in);
conv1 runs k-outer so its first matmul only needs the first 128-channel
chunk. Weights are single consolidated DMAs (w1 interleaved after the
first x half on the SP HWDGE queue; biases via the GpSimd SWDGE queue).
A short run of dummy matmuls on zeroed tiles warms the PE p-state ramp
while the first x DMA flies. conv3 (1x1) and the stride-2 shortcut conv
accumulate into the same PSUM tile; shortcut weights are pre-scaled by
(css/c3s) on the host so both contributions share one output scale. The
last pair stores per-m with a fused FEPI epilogue to shorten the tail.

Engine balance: PE does matmuls (~76us of 88us sim), DVE does the
x-quant pair + epilogue rounding ts ops (~28us), ACT does epilogue
relu+bias + conv3 relu (~37us), Pool (GpSimd) does conv2 pad memsets +
the final min(.,6) (~26us).
"""

import sys
from contextlib import ExitStack

import numpy as np

sys.path.insert(0, "/opt/trn_rl_repo")

import ml_dtypes  # noqa: E402

import concourse.bacc as bacc  # noqa: E402
import concourse.bass as bass  # noqa: E402
import concourse.dve_ops as dve_ops  # noqa: E402
import concourse.tile as tile  # noqa: E402
from concourse import mybir  # noqa: E402
from concourse.bass_utils import run_bass_kernel_spmd  # noqa: E402
from concourse.dve_spec import (  # noqa: E402
    C0 as DC0,
    C1 as DC1,
    C2 as DC2,
    One as DOne,
    Spec,
    Src0 as DSrc0,
    Src1 as DSrc1,
    _has_src1,
    lower as dve_lower,
    maxx,
    minn,
    relu as drelu,
)
from concourse.dve_uop import DveOpSpec  # noqa: E402
from concourse.dve_table_gen import dve_ver_for  # noqa: E402
from concourse.dve_ops import DveOp  # noqa: E402

F32 = mybir.dt.float32
F16 = mybir.dt.float16
BF16 = mybir.dt.bfloat16
ALU = mybir.AluOpType
AFT = mybir.ActivationFunctionType
BF16NP = ml_dtypes.bfloat16

C_MAGIC = float(np.float32(12582912.0))  # 1.5 * 2**23
XOFF = 383.0  # x-quant code offset: 256 (bf16 rounding binade) + 127

N_CORES = 8
B_LOC = 8  # images per core
PAIRS = B_LOC // 2


def _register_dve_op(name, spec, subdim=False):
    """Register a custom DVE op at runtime (table is generated per-NEFF)."""
    for o in dve_ops.OPS:
        if o.name == name:
            return o
    row = dve_ops._CUSTOM_DVE_ROW_BASE + len(dve_ops.OPS)
    assert row < 0x20
    shas = {}
    for ver in ("v3", "v4"):
        tmp = DveOpSpec(
            name=name, opcode=row, uops=dve_lower(spec, ver=ver),
            rd1_en=_has_src1(spec),
        )
        shas[ver] = tmp.sha(ver)
    op = DveOp(name, spec, subdim=subdim, uops_sha=shas)
    dve_ops.OPS.append(op)
    dve_ops._SUB_OPCODE_FOR_NAME[name] = row
    dve_ops.CUSTOM_DVE_SPECS[name] = spec
    return op


def _b(in0, in1):
    """Sim helper: in1 streams elementwise on HW; align shapes for numpy."""
    if isinstance(in1, np.ndarray) and in1.size == in0.size:
        return in1.reshape(in0.shape)
    return in1


# quantize epilogue: v = in0*alpha + beta ; out = clip(round(v),0,127) + 128
# round via magic-add: u = (v + C) rounds to integer grid; clip in shifted
# domain [C, C+127]; subtract C-128.
QEPI = _register_dve_op(
    "BNECK_QEPI_ANT",
    Spec(
        body=(minn(maxx((DSrc0 * DC0 + DSrc1) + DC1, DC1), DC1 + DC2) - DC1)
        + (DC2 + DOne),
        reference=lambda in0, in1, s0, s1, imm2: np.minimum(
            np.maximum(np.round(in0 * s0 + _b(in0, in1)), 0.0), imm2
        )
        + (imm2 + 1.0),
    ),
)


# final epilogue (tail): out = min(relu(in0*gamma + delta), 6)
FEPI = _register_dve_op(
    "BNECK_FEPI_ANT",
    Spec(
        body=minn(drelu(DSrc0 * DC0 + DSrc1), DC1),
        reference=lambda in0, in1, s0, s1, imm2: np.minimum(
            np.maximum(in0 * s0 + _b(in0, in1), 0.0), s1
        ),
    ),
)


def _build_nc(pairs=PAIRS):
    nc = bacc.Bacc("TRN2", target_bir_lowering=False, debug=False)
    b_loc = 2 * pairs

    x_d = nc.dram_tensor("x", [b_loc, 4, 128, 784], F16, kind="ExternalInput")
    w1_d = nc.dram_tensor("w1l", [4, 128, 256], BF16, kind="ExternalInput")
    w2_d = nc.dram_tensor("w2l", [9, 2, 128, 256], BF16, kind="ExternalInput")
    w3_d = nc.dram_tensor("w3l", [2, 128, 1024], BF16, kind="ExternalInput")
    ws_d = nc.dram_tensor("wsl", [4, 128, 1024], BF16, kind="ExternalInput")
    b1_d = nc.dram_tensor("beta1", [128, 2], F32, kind="ExternalInput")
    b2_d = nc.dram_tensor("beta2", [128, 2], F32, kind="ExternalInput")
    dl_d = nc.dram_tensor("delta", [128, 8], F32, kind="ExternalInput")
    # packed [pair, m-pair, q, (m2 i h)] to keep the out-DMA 2D; host unpacks
    y_d = nc.dram_tensor("y", [pairs, 4, 128, 784], BF16, kind="ExternalOutput")

    # scales (filled per-call via globals set by kernel(); see _SCALES)
    a1, a2, g3 = _SCALES

    with tile.TileContext(nc) as tc, ExitStack() as ctx:
        wp = ctx.enter_context(tc.tile_pool(name="w", bufs=1))
        xinp = ctx.enter_context(tc.tile_pool(name="xin", bufs=4))
        yap = ctx.enter_context(tc.tile_pool(name="ya", bufs=4))
        xqp = ctx.enter_context(tc.tile_pool(name="xq", bufs=3))
        p2p = ctx.enter_context(tc.tile_pool(name="p2", bufs=2))
        t3p = ctx.enter_context(tc.tile_pool(name="t3", bufs=2))
        rp = ctx.enter_context(tc.tile_pool(name="r", bufs=5))
        yop = ctx.enter_context(tc.tile_pool(name="yo", bufs=4))
        pc1 = ctx.enter_context(tc.tile_pool(name="pc1", bufs=3, space="PSUM"))
        pc2 = ctx.enter_context(tc.tile_pool(name="pc2", bufs=2, space="PSUM"))
        pc3 = ctx.enter_context(tc.tile_pool(name="pc3", bufs=3, space="PSUM"))

        # ---- weights + biases (loaded once) ----
        # w1 is one consolidated DMA, interleaved after the first x half
        # on the SP queue (see load_w1 callback); biases ride the SWDGE
        # queue so their generation never blocks the x stream
        w1t = wp.tile([128, 4, 256], BF16, tag="w1t")

        def load_w1():
            nc.sync.dma_start(w1t[:], w1_d.rearrange("k q n -> q k n"))

        beta1 = wp.tile([128, 2], F32, tag="beta1")
        nc.gpsimd.dma_start(beta1[:], b1_d[:])
        c127 = wp.tile([128, 1], F32, tag="c127")
        nc.vector.memset(c127[:], 127.0)
        # preload the ACT function table (Relu) while the first DMAs fly
        warm_act = wp.tile([128, 1], F32, tag="warm_act")
        nc.scalar.activation(warm_act[:], c127[:], AFT.Relu, bias=c127[:], scale=1.0)
        # warm the PE p-state ramp with dummy matmuls on zeroed tiles while
        # the first x chunks are still in flight
        wz = wp.tile([128, 128], F16, tag="wz")
        rz = wp.tile([128, 392], F16, tag="rz")
        nc.vector.memset(rz[:], 0.0)
        nc.vector.memset(wz[:], 0.0)
        pz = pc1.tile([128, 392], F32, tag="ps1", name="pz")
        for _ in range(10):
            nc.tensor.matmul(pz[:], wz[:], rz[:], start=True, stop=True)

        st = {}  # per-pair tiles: xq, p2, t3

        def emit_xload(p, split_first=False, after_first=None, fine=False):
            """One DMA per image into a per-pair [q, i, k, h] tile, then
            quantize in [128, 1568] slices (two k chunks at a time). For
            pair 0 (split_first) each image arrives in two half-DMAs and
            image 0 quantizes per-k, so conv1 can start sooner."""
            xq = xqp.tile([128, 2, 4, 784], BF16, tag="xq", name=f"xq_{p}")
            st[p] = {"xq": xq}
            xin = xinp.tile([128, 2, 4, 784], F16, tag="xin", name=f"xin_{p}")
            for i in (0, 1):
                if split_first:
                    for kk in (0, 2):
                        nc.sync.dma_start(
                            xin[:, i, kk : kk + 2],
                            x_d[2 * p + i, kk : kk + 2].rearrange(
                                "k q h -> q k h"
                            ),
                        )
                        if i == 0 and kk == 0 and after_first is not None:
                            after_first()
                else:
                    nc.sync.dma_start(
                        xin[:, i], x_d[2 * p + i].rearrange("k q h -> q k h")
                    )
                nk = 1 if (fine or (split_first and i == 0)) else 2
                for kk in range(0, 4, nk):
                    ya = yap.tile([128, 1568], BF16, tag="ya", name="ya")
                    yav = ya[:, : nk * 784].rearrange("q (a b) -> q a b", a=nk)
                    xqv = xq[:, i, kk : kk + nk]
                    # x-quant as two 4x-mode DVE tensor_scalars (all
                    # operands 2-byte, packed, SBUF): the bf16 output
                    # convert of op 1 rounds RNE on the integer grid
                    # (ulp == 1 in [256, 512)); op 2 clamps to the code
                    # range. codes+383 = clip(rne(254x + 383), 256, 510);
                    # out-of-binade values land outside [256, 510] and are
                    # fixed by the clamps, so this equals
                    # clip(rne(254x), -127, 127) + 383 exactly.
                    nc.vector.tensor_scalar(
                        yav, xin[:, i, kk : kk + nk],
                        254.0, 383.0, op0=ALU.mult, op1=ALU.add,
                    )
                    nc.vector.tensor_scalar(
                        xqv, yav, 256.0, 510.0, op0=ALU.max, op1=ALU.min,
                    )

        def emit_conv1(p):
            xq = st[p]["xq"]
            p2 = []
            for m in range(2):
                # pad value 128 == quantized zero in the +128-shifted domain.
                # Only the cells conv2 actually reads need padding: row 0
                # and column 1 of each 29x32 image plane.
                t = p2p.tile([128, 1856], BF16, tag=f"p2_{m}")
                pv = t.rearrange("q (i r c) -> q i r c", i=2, r=29, c=32)
                nc.gpsimd.memset(pv[:, :, 0, :], 384.0)
                nc.gpsimd.memset(pv[:, :, 1:29, 1], 384.0)
                p2.append(t)
            for i in (0, 1):
                ps = {}
                # k-outer for three groups from pc1 (3 banks); the fourth
                # group borrows a pc3 bank for pair 0 (conv3 is far away,
                # so the bank is free) to stay k-outer during the x stream,
                # and runs k-inner for later pairs (chunks are prefetched)
                quads = [(0, 0), (0, 1), (1, 0)]
                if p <= 1:
                    quads.append((1, 1))
                    ps[(1, 1)] = pc3.tile(
                        [128, 392], F32, name="ps1b", tag="ps3"
                    )
                for k in range(4):
                    for m, hf in quads:
                        if k == 0 and (m, hf) != (1, 1):
                            ps[(m, hf)] = pc1.tile(
                                [128, 392], F32,
                                name=f"ps1_{m}_{hf}", tag="ps1",
                            )
                        nc.tensor.matmul(
                            ps[(m, hf)][:],
                            w1t[:, k, m * 128 : (m + 1) * 128],
                            xq[:, i, k, hf * 392 : hf * 392 + 392],
                            start=(k == 0),
                            stop=(k == 3),
                            skip_group_check=True,
                        )
                if p > 1:
                    ps[(1, 1)] = pc1.tile(
                        [128, 392], F32, name="ps1_1_1", tag="ps1"
                    )
                    for k in range(4):
                        nc.tensor.matmul(
                            ps[(1, 1)][:],
                            w1t[:, k, 128:256],
                            xq[:, i, k, 392:784],
                            start=(k == 0),
                            stop=(k == 3),
                            skip_group_check=True,
                        )
                for m in (0, 1):
                    pv = p2[m].rearrange("q (i r c) -> q i r c", i=2, r=29, c=32)
                    for hf in (0, 1):
                        # epilogue: ACT clips low (relu of a1*ps + beta),
                        # DVE ts clips high and shifts by +384 -- the bf16
                        # convert rounds RNE on the integer grid in
                        # [256, 512), and 384 is even so tie parity matches
                        # jnp.round exactly
                        t1 = rp.tile([128, 392], F32, tag="t1", name="t1")
                        nc.scalar.activation(
                            t1[:], ps[(m, hf)][:], AFT.Relu,
                            bias=beta1[:, m : m + 1], scale=a1,
                        )
                        nc.vector.tensor_scalar(
                            pv[:, i, 1 + 14 * hf : 15 + 14 * hf, 2:30],
                            t1[:].rearrange("q (a b) -> q a b", a=14),
                            127.25, 384.0, op0=ALU.min, op1=ALU.add,
                        )
            st[p]["p2"] = p2

        def emit_conv2(p):
            p2 = st[p]["p2"]
            t3 = []
            for m in range(2):
                ps2 = pc2.tile([128, 392], F32)
                first = True
                for k in range(2):
                    pv = p2[k].rearrange("q (i r c) -> q i r c", i=2, r=29, c=32)
                    for tp in range(9):
                        ky, kx = divmod(tp, 3)
                        nc.tensor.matmul(
                            ps2[:],
                            w2t[:, tp, k, m * 128 : (m + 1) * 128],
                            pv[:, :, ky : min(ky + 28, 29) : 2, 1 + kx : 29 + kx : 2],
                            start=first,
                            stop=(k == 1 and tp == 8),
                        )
                        first = False
                t3m = t3p.tile([128, 392], BF16, tag=f"t3_{m}")
                t2 = rp.tile([128, 392], F32, tag="t1", name="t2")
                nc.scalar.activation(
                    t2[:], ps2[:], AFT.Relu, bias=beta2[:, m : m + 1], scale=a2
                )
                nc.vector.tensor_scalar(
                    t3m[:], t2[:], 127.25, 384.0, op0=ALU.min, op1=ALU.add
                )
                t3.append(t3m)
            st[p]["t3"] = t3

        def emit_conv3(p, last=False):
            xq, t3 = st[p]["xq"], st[p]["t3"]
            yo = None
            for m in range(8):
                ps3 = pc3.tile([128, 392], F32)
                # shortcut first: only needs xq, giving t3's epilogue time
                for k in range(4):
                    xv = xq[:, :, k].rearrange("q i (r c) -> q i r c", r=28, c=28)
                    nc.tensor.matmul(
                        ps3[:],
                        wst[:, k, m * 128 : (m + 1) * 128],
                        xv[:, :, 0:28:2, 0:28:2],
                        start=(k == 0),
                        stop=False,
                        skip_group_check=True,
                    )
                for k in range(2):
                    nc.tensor.matmul(
                        ps3[:],
                        w3t[:, k, m * 128 : (m + 1) * 128],
                        t3[k][:],
                        start=False,
                        stop=(k == 1),
                        skip_group_check=True,
                    )
                if last and m >= 6:
                    # tail: single fused DVE epilogue + per-m store so the
                    # final chain after the last matmul is as short as possible
                    yo = yop.tile([128, 392], BF16, tag="yot", name="yot")
                    nc.vector._custom_dve(
                        FEPI, out=yo[:], in0=ps3[:], in1=dlf[m - 6][:],
                        s0=g3, s1=6.0,
                    )
                    nc.sync.dma_start(
                        y_d[p, m // 2][:, (m % 2) * 392 : (m % 2) * 392 + 392],
                        yo[:],
                    )
                    continue
                r3 = rp.tile([128, 392], F32, tag="r3")
                nc.scalar.activation(
                    r3[:], ps3[:], AFT.Relu, bias=delta[:, m : m + 1], scale=g3
                )
                if m % 2 == 0:
                    yo = yop.tile([128, 784], BF16)
                nc.gpsimd.tensor_scalar(
                    yo[:, (m % 2) * 392 : (m % 2) * 392 + 392],
                    r3[:], 6.0, None, op0=ALU.min,
                )
                if m % 2 == 1:
                    nc.sync.dma_start(y_d[p, m // 2], yo[:])
            del st[p]

        # pair 0+1 x loads first (SP queue), then pair-0 conv1
        emit_xload(0, split_first=True, after_first=load_w1)
        emit_xload(1, fine=True)
        emit_conv1(0)

        # bulk weights: one consolidated DMA per tensor on the SP queue,
        # small biases via the GpSimd SWDGE queue
        w2t = wp.tile([128, 9, 2, 256], BF16, tag="w2t")
        nc.sync.dma_start(w2t[:], w2_d.rearrange("t k q n -> q t k n"))
        wst = wp.tile([128, 4, 1024], BF16, tag="wst")
        nc.sync.dma_start(wst[:], ws_d.rearrange("k q n -> q k n"))
        w3t = wp.tile([128, 2, 1024], BF16, tag="w3t")
        nc.sync.dma_start(w3t[:], w3_d.rearrange("k q n -> q k n"))
        beta2 = wp.tile([128, 2], F32, tag="beta2")
        nc.gpsimd.dma_start(beta2[:], b2_d[:])
        delta = wp.tile([128, 8], F32, tag="delta")
        nc.gpsimd.dma_start(delta[:], dl_d[:])
        dlf = []
        for j in (6, 7):
            t = wp.tile([128, 392], F32, tag=f"dlf{j}", name=f"dlf{j}")
            nc.gpsimd.tensor_copy(t[:], delta[:, j : j + 1].to_broadcast((128, 392)))
            dlf.append(t)

        # software pipeline: conv2/conv3 of pair p-1 run under conv1 of pair p
        for p in range(1, pairs):
            emit_conv1(p)
            emit_conv2(p - 1)
            if p + 1 < pairs:
                emit_xload(p + 1)
            emit_conv3(p - 1)
        emit_conv2(pairs - 1)
        emit_conv3(pairs - 1, last=True)
    return nc


_SCALES = (1.0, 1.0, 1.0)


def _prep(w1, b1, w2, b2, w3, b3, wsw, bs):
    """Host-side weight quantization + constant folding (all tiny tensors)."""
    f32 = np.float32

    def qw(w):
        s = f32(np.max(np.abs(w)))
        wq = np.round(np.clip(w / s, f32(-1.0), f32(1.0)) * f32(127.0)).astype(
            np.float32
        )
        return wq, s

    def qb(b):
        return np.round(b * f32(127.0)).astype(np.float32)

    w1q, c1s = qw(w1)  # [256,512,1,1]
    w2q, c2s = qw(w2)  # [256,256,3,3]
    w3q, c3s = qw(w3)  # [1024,256,1,1]
    wsq, css = qw(wsw)  # [1024,512,1,1]
    B1, B2, B3, Bs = qb(b1), qb(b2), qb(b3), qb(bs)

    a1 = f32(2.0) * c1s / f32(127.0)
    a2 = f32(2.0) * c2s / f32(127.0)
    g3 = c3s / f32(2.0 * 16129.0)
    rho = css / c3s

    # lhsT layouts
    w1l = np.ascontiguousarray(
        w1q[:, :, 0, 0].T.reshape(4, 128, 256).astype(BF16NP)
    )
    # w2 taps: [ky,kx] -> lhsT [cin, cout] per tap
    w2l = np.ascontiguousarray(
        w2q.transpose(2, 3, 1, 0).reshape(9, 2, 128, 256).astype(BF16NP)
    )
    w3l = np.ascontiguousarray(
        w3q[:, :, 0, 0].T.reshape(2, 128, 1024).astype(BF16NP)
    )
    ws_sc = (rho * wsq[:, :, 0, 0]).astype(BF16NP)  # [1024,512] scaled bf16
    wsl = np.ascontiguousarray(ws_sc.T.reshape(4, 128, 1024))

    # column sums for the activation offset corrections (fp64 exact)
    K1 = w1q[:, :, 0, 0].astype(np.float64).sum(axis=1)  # [256]
    K2 = w2q.astype(np.float64).sum(axis=(1, 2, 3))  # [256]
    K3 = w3q[:, :, 0, 0].astype(np.float64).sum(axis=1)  # [1024]
    Ks = ws_sc.astype(np.float64).sum(axis=1)  # [1024]

    # activation shifts: x-quant adds +1151 (f16 trick), conv epis add +128
    beta1 = (
        f32(4.0) * B1
        - (a1.astype(np.float64) * 383.0 * K1).astype(np.float32)
    ).astype(np.float32)
    beta2 = (f32(4.0) * B2 - a2 * f32(384.0) * K2.astype(np.float32)).astype(
        np.float32
    )
    delta0 = B3 * c3s / (f32(127.0) * c2s) + Bs / f32(127.0)
    delta = (
        delta0
        - (g3.astype(np.float64) * (384.0 * K3 + 383.0 * Ks)).astype(np.float32)
    ).astype(np.float32)

    beta1 = np.ascontiguousarray(beta1.reshape(2, 128).T)  # [128,2]
    beta2 = np.ascontiguousarray(beta2.reshape(2, 128).T)
    delta = np.ascontiguousarray(delta.reshape(8, 128).T)  # [128,8]

    return dict(
        w1l=w1l, w2l=w2l, w3l=w3l, wsl=wsl,
        beta1=beta1, beta2=beta2, delta=delta,
        a1=float(a1), a2=float(a2), g3=float(g3),
    )


def kernel(x, w1, b1, w2, b2, w3, b3, ws, bs):
    global _SCALES
    x16 = np.asarray(x).astype(np.float16)
    pre = _prep(
        np.asarray(w1, np.float32), np.asarray(b1, np.float32),
        np.asarray(w2, np.float32), np.asarray(b2, np.float32),
        np.asarray(w3, np.float32), np.asarray(b3, np.float32),
        np.asarray(ws, np.float32), np.asarray(bs, np.float32),
    )
    _SCALES = (pre["a1"], pre["a2"], pre["g3"])
    nc = _build_nc()
    nc.compile()

    shared = {
        "w1l": pre["w1l"], "w2l": pre["w2l"], "w3l": pre["w3l"],
        "wsl": pre["wsl"], "beta1": pre["beta1"], "beta2": pre["beta2"],
        "delta": pre["delta"],
    }
    in_maps = []
    for c in range(N_CORES):
        xs = np.ascontiguousarray(
            x16[c * B_LOC : (c + 1) * B_LOC].reshape(B_LOC, 4, 128, 784)
        )
        in_maps.append({"x": xs, **shared})

    import os

    tmpdir = os.environ.get("KERNEL_TRACE_DIR") or None
    if tmpdir:
        os.makedirs(tmpdir, exist_ok=True)
    res = run_bass_kernel_spmd(nc, in_maps, list(range(N_CORES)), tmpdir=tmpdir)
    global LAST_RESULT
    LAST_RESULT = res
    outs = [unpack_y(res.results[c]["y"]) for c in range(N_CORES)]
    return np.ascontiguousarray(np.concatenate(outs, axis=0))


def unpack_y(y):
    """[pairs,4,128,784] packed -> [2*pairs, 1024, 14, 14]."""
    p = y.shape[0]
    y = y.reshape(p, 4, 128, 2, 2, 196)  # (p, mp, q, m2, i, h)
    y = y.transpose(0, 4, 1, 3, 2, 5)  # (p, i, mp, m2, q, h)
    return np.ascontiguousarray(
        y.reshape(2 * p, 1024, 14, 14).astype(np.float32)
    )



# revision 6
# speedup vs baseline: 2.1200x; 2.1200x over previous
"""Trainium2 Bass kernel for the quantized ResNet Bottleneck block.

Strategy
--------
Data parallel over batch: 64 images -> 8 cores x 8 images. Each core runs an
identical Bass program; weights are replicated.

All convs are executed as integer-valued matmuls accumulated in fp32 PSUM
(exact: quantized codes are integers; offsets pass through each conv as
per-output-channel constants folded into the next bias on the host).

Numeric tricks (all exact up to tie-perturbations far below budget):
  * x-quant: two 4x-mode DVE tensor_scalars on the f16 input. Op 1
    computes 254*x + 383 with bf16 output -- in [256, 512) the bf16 ulp
    is 1, so the convert rounds RNE to the integer grid; op 2 clamps to
    [256, 510]. Out-of-binade values land outside the clamp range, so
    this equals clip(rne(254x), -127, 127) + 383 exactly.
  * conv1/conv2 epilogues: ACT computes relu(a*psum + beta) (beta read
    per-partition, no broadcast tiles); a DVE tensor_scalar then does
    (min 127.25) + 384 with bf16 output -- same ulp-1 rounding, and 384
    is even so tie parity matches jnp.round. Codes live in [384, 511]
    (exact in bf16); offsets fold into the next layer's bias via weight
    column sums (x: 383; p2/t3 and the conv2 pad value: 384).

Schedule: x ships from host as f16 (halves input DMA; rel-err impact
~2.6e-3, budget 2e-2) and y returns as bf16 (host upcasts; ~0.2% rel).
x loads are one DMA per image into a per-pair [q, i, k, h] tile (pair 0:
split into half-DMAs + finer quant slices so conv1 starts ~4us in);
conv1 runs k-outer so its first matmul only needs the first 128-channel
chunk. Weights are single consolidated DMAs (w1 interleaved after the
first x half on the SP HWDGE queue; biases via the GpSimd SWDGE queue).
A short run of dummy matmuls on zeroed tiles warms the PE p-state ramp
while the first x DMA flies. conv3 (1x1) and the stride-2 shortcut conv
accumulate into the same PSUM tile; shortcut weights are pre-scaled by
(css/c3s) on the host so both contributions share one output scale. The
last pair stores per-m with a fused FEPI epilogue to shorten the tail.

Engine balance: PE does matmuls (~76us of 88us sim), DVE does the
x-quant pair + epilogue rounding ts ops (~28us), ACT does epilogue
relu+bias + conv3 relu (~37us), Pool (GpSimd) does conv2 pad memsets +
the final min(.,6) (~26us).
"""

import sys
from contextlib import ExitStack

import numpy as np

sys.path.insert(0, "/opt/trn_rl_repo")

import ml_dtypes  # noqa: E402

import concourse.bacc as bacc  # noqa: E402
import concourse.bass as bass  # noqa: E402
import concourse.dve_ops as dve_ops  # noqa: E402
import concourse.tile as tile  # noqa: E402
from concourse import mybir  # noqa: E402
from concourse.bass_utils import run_bass_kernel_spmd  # noqa: E402
from concourse.dve_spec import (  # noqa: E402
    C0 as DC0,
    C1 as DC1,
    C2 as DC2,
    One as DOne,
    Spec,
    Src0 as DSrc0,
    Src1 as DSrc1,
    _has_src1,
    lower as dve_lower,
    maxx,
    minn,
    relu as drelu,
)
from concourse.dve_uop import DveOpSpec  # noqa: E402
from concourse.dve_table_gen import dve_ver_for  # noqa: E402
from concourse.dve_ops import DveOp  # noqa: E402

F32 = mybir.dt.float32
F16 = mybir.dt.float16
BF16 = mybir.dt.bfloat16
ALU = mybir.AluOpType
AFT = mybir.ActivationFunctionType
BF16NP = ml_dtypes.bfloat16

C_MAGIC = float(np.float32(12582912.0))  # 1.5 * 2**23
XOFF = 383.0  # x-quant code offset: 256 (bf16 rounding binade) + 127

N_CORES = 8
B_LOC = 8  # images per core
PAIRS = B_LOC // 2


def _register_dve_op(name, spec, subdim=False):
    """Register a custom DVE op at runtime (table is generated per-NEFF)."""
    for o in dve_ops.OPS:
        if o.name == name:
            return o
    row = dve_ops._CUSTOM_DVE_ROW_BASE + len(dve_ops.OPS)
    assert row < 0x20
    shas = {}
    for ver in ("v3", "v4"):
        tmp = DveOpSpec(
            name=name, opcode=row, uops=dve_lower(spec, ver=ver),
            rd1_en=_has_src1(spec),
        )
        shas[ver] = tmp.sha(ver)
    op = DveOp(name, spec, subdim=subdim, uops_sha=shas)
    dve_ops.OPS.append(op)
    dve_ops._SUB_OPCODE_FOR_NAME[name] = row
    dve_ops.CUSTOM_DVE_SPECS[name] = spec
    return op


def _b(in0, in1):
    """Sim helper: in1 streams elementwise on HW; align shapes for numpy."""
    if isinstance(in1, np.ndarray) and in1.size == in0.size:
        return in1.reshape(in0.shape)
    return in1


# quantize epilogue: v = in0*alpha + beta ; out = clip(round(v),0,127) + 128
# round via magic-add: u = (v + C) rounds to integer grid; clip in shifted
# domain [C, C+127]; subtract C-128.
QEPI = _register_dve_op(
    "BNECK_QEPI_ANT",
    Spec(
        body=(minn(maxx((DSrc0 * DC0 + DSrc1) + DC1, DC1), DC1 + DC2) - DC1)
        + (DC2 + DOne),
        reference=lambda in0, in1, s0, s1, imm2: np.minimum(
            np.maximum(np.round(in0 * s0 + _b(in0, in1)), 0.0), imm2
        )
        + (imm2 + 1.0),
    ),
)


# final epilogue (tail): out = min(relu(in0*gamma + delta), 6)
FEPI = _register_dve_op(
    "BNECK_FEPI_ANT",
    Spec(
        body=minn(drelu(DSrc0 * DC0 + DSrc1), DC1),
        reference=lambda in0, in1, s0, s1, imm2: np.minimum(
            np.maximum(in0 * s0 + _b(in0, in1), 0.0), s1
        ),
    ),
)


def _build_nc(pairs=PAIRS):
    nc = bacc.Bacc("TRN2", target_bir_lowering=False, debug=False)
    b_loc = 2 * pairs

    x_d = nc.dram_tensor("x", [b_loc, 4, 128, 784], F16, kind="ExternalInput")
    w1_d = nc.dram_tensor("w1l", [4, 128, 256], BF16, kind="ExternalInput")
    w2_d = nc.dram_tensor("w2l", [9, 2, 128, 256], BF16, kind="ExternalInput")
    w3_d = nc.dram_tensor("w3l", [2, 128, 1024], BF16, kind="ExternalInput")
    ws_d = nc.dram_tensor("wsl", [4, 128, 1024], BF16, kind="ExternalInput")
    b1_d = nc.dram_tensor("beta1", [128, 2], F32, kind="ExternalInput")
    b2_d = nc.dram_tensor("beta2", [128, 2], F32, kind="ExternalInput")
    dl_d = nc.dram_tensor("delta", [128, 8], F32, kind="ExternalInput")
    # packed [pair, m-pair, q, (m2 i h)] to keep the out-DMA 2D; host unpacks
    y_d = nc.dram_tensor("y", [pairs, 4, 128, 784], BF16, kind="ExternalOutput")

    # scales (filled per-call via globals set by kernel(); see _SCALES)
    a1, a2, g3 = _SCALES

    with tile.TileContext(nc) as tc, ExitStack() as ctx:
        wp = ctx.enter_context(tc.tile_pool(name="w", bufs=1))
        xinp = ctx.enter_context(tc.tile_pool(name="xin", bufs=4))
        yap = ctx.enter_context(tc.tile_pool(name="ya", bufs=4))
        xqp = ctx.enter_context(tc.tile_pool(name="xq", bufs=3))
        p2p = ctx.enter_context(tc.tile_pool(name="p2", bufs=2))
        t3p = ctx.enter_context(tc.tile_pool(name="t3", bufs=2))
        rp = ctx.enter_context(tc.tile_pool(name="r", bufs=5))
        yop = ctx.enter_context(tc.tile_pool(name="yo", bufs=4))
        pc1 = ctx.enter_context(tc.tile_pool(name="pc1", bufs=3, space="PSUM"))
        pc2 = ctx.enter_context(tc.tile_pool(name="pc2", bufs=2, space="PSUM"))
        pc3 = ctx.enter_context(tc.tile_pool(name="pc3", bufs=3, space="PSUM"))

        # ---- weights + biases (loaded once) ----
        # w1 is one consolidated DMA, interleaved after the first x half
        # on the SP queue (see load_w1 callback); biases ride the SWDGE
        # queue so their generation never blocks the x stream
        w1t = wp.tile([128, 4, 256], BF16, tag="w1t")

        def load_w1():
            nc.sync.dma_start(w1t[:], w1_d.rearrange("k q n -> q k n"))

        beta1 = wp.tile([128, 2], F32, tag="beta1")
        nc.gpsimd.dma_start(beta1[:], b1_d[:])
        c127 = wp.tile([128, 1], F32, tag="c127")
        nc.vector.memset(c127[:], 127.0)
        # preload the ACT function table (Relu) while the first DMAs fly
        warm_act = wp.tile([128, 1], F32, tag="warm_act")
        nc.scalar.activation(warm_act[:], c127[:], AFT.Relu, bias=c127[:], scale=1.0)
        # warm the PE p-state ramp with dummy matmuls on zeroed tiles while
        # the first x chunks are still in flight
        wz = wp.tile([128, 128], F16, tag="wz")
        rz = wp.tile([128, 392], F16, tag="rz")
        nc.vector.memset(rz[:], 0.0)
        nc.vector.memset(wz[:], 0.0)
        pz = pc1.tile([128, 392], F32, tag="ps1", name="pz")
        for _ in range(10):
            nc.tensor.matmul(pz[:], wz[:], rz[:], start=True, stop=True)

        st = {}  # per-pair tiles: xq, p2, t3

        def emit_xload(p, split_first=False, after_first=None, fine=False):
            """One DMA per image into a per-pair [q, i, k, h] tile, then
            quantize in [128, 1568] slices (two k chunks at a time). For
            pair 0 (split_first) each image arrives in two half-DMAs and
            image 0 quantizes per-k, so conv1 can start sooner."""
            xq = xqp.tile([128, 2, 4, 784], BF16, tag="xq", name=f"xq_{p}")
            st[p] = {"xq": xq}
            xin = xinp.tile([128, 2, 4, 784], F16, tag="xin", name=f"xin_{p}")
            for i in (0, 1):
                if split_first:
                    for kk in (0, 2):
                        nc.sync.dma_start(
                            xin[:, i, kk : kk + 2],
                            x_d[2 * p + i, kk : kk + 2].rearrange(
                                "k q h -> q k h"
                            ),
                        )
                        if i == 0 and kk == 0 and after_first is not None:
                            after_first()
                else:
                    nc.sync.dma_start(
                        xin[:, i], x_d[2 * p + i].rearrange("k q h -> q k h")
                    )
                nk = 1 if (fine or (split_first and i == 0)) else 2
                for kk in range(0, 4, nk):
                    ya = yap.tile([128, 1568], BF16, tag="ya", name="ya")
                    # strictly-2D contiguous APs: 3-dim views drop the DVE
                    # to 1 elem/cycle on HW
                    yav = ya[:, : nk * 784]
                    xqv = xq[:, i, kk : kk + nk].rearrange("q a b -> q (a b)")
                    # x-quant as two 4x-mode DVE tensor_scalars (all
                    # operands 2-byte, packed, SBUF): the bf16 output
                    # convert of op 1 rounds RNE on the integer grid
                    # (ulp == 1 in [256, 512)); op 2 clamps to the code
                    # range. codes+383 = clip(rne(254x + 383), 256, 510);
                    # out-of-binade values land outside [256, 510] and are
                    # fixed by the clamps, so this equals
                    # clip(rne(254x), -127, 127) + 383 exactly.
                    nc.vector.tensor_scalar(
                        yav,
                        xin[:, i, kk : kk + nk].rearrange("q a b -> q (a b)"),
                        254.0, 383.0, op0=ALU.mult, op1=ALU.add,
                    )
                    nc.vector.tensor_scalar(
                        xqv, yav, 256.0, 510.0, op0=ALU.max, op1=ALU.min,
                    )

        def emit_conv1(p):
            xq = st[p]["xq"]
            p2 = []
            for m in range(2):
                # pad value 128 == quantized zero in the +128-shifted domain.
                # Only the cells conv2 actually reads need padding: row 0
                # and column 1 of each 29x32 image plane.
                t = p2p.tile([128, 1856], BF16, tag=f"p2_{m}")
                pv = t.rearrange("q (i r c) -> q i r c", i=2, r=29, c=32)
                nc.gpsimd.memset(pv[:, :, 0, :], 384.0)
                nc.gpsimd.memset(pv[:, :, 1:29, 1], 384.0)
                p2.append(t)
            for i in (0, 1):
                ps = {}
                # k-outer for three groups from pc1 (3 banks); the fourth
                # group borrows a pc3 bank for pair 0 (conv3 is far away,
                # so the bank is free) to stay k-outer during the x stream,
                # and runs k-inner for later pairs (chunks are prefetched)
                quads = [(0, 0), (0, 1), (1, 0)]
                if p <= 1:
                    quads.append((1, 1))
                    ps[(1, 1)] = pc3.tile(
                        [128, 392], F32, name="ps1b", tag="ps3"
                    )
                for k in range(4):
                    for m, hf in quads:
                        if k == 0 and (m, hf) != (1, 1):
                            ps[(m, hf)] = pc1.tile(
                                [128, 392], F32,
                                name=f"ps1_{m}_{hf}", tag="ps1",
                            )
                        nc.tensor.matmul(
                            ps[(m, hf)][:],
                            w1t[:, k, m * 128 : (m + 1) * 128],
                            xq[:, i, k, hf * 392 : hf * 392 + 392],
                            start=(k == 0),
                            stop=(k == 3),
                            skip_group_check=True,
                        )
                if p > 1:
                    ps[(1, 1)] = pc1.tile(
                        [128, 392], F32, name="ps1_1_1", tag="ps1"
                    )
                    for k in range(4):
                        nc.tensor.matmul(
                            ps[(1, 1)][:],
                            w1t[:, k, 128:256],
                            xq[:, i, k, 392:784],
                            start=(k == 0),
                            stop=(k == 3),
                            skip_group_check=True,
                        )
                for m in (0, 1):
                    pv = p2[m].rearrange("q (i r c) -> q i r c", i=2, r=29, c=32)
                    for hf in (0, 1):
                        # epilogue: ACT clips low (relu of a1*ps + beta),
                        # DVE ts clips high and shifts by +384 -- the bf16
                        # convert rounds RNE on the integer grid in
                        # [256, 512), and 384 is even so tie parity matches
                        # jnp.round exactly
                        t1 = rp.tile([128, 392], F32, tag="t1", name="t1")
                        nc.scalar.activation(
                            t1[:], ps[(m, hf)][:], AFT.Relu,
                            bias=beta1[:, m : m + 1], scale=a1,
                        )
                        nc.vector.tensor_scalar(
                            pv[:, i, 1 + 14 * hf : 15 + 14 * hf, 2:30],
                            t1[:].rearrange("q (a b) -> q a b", a=14),
                            127.25, 384.0, op0=ALU.min, op1=ALU.add,
                        )
            st[p]["p2"] = p2

        def emit_conv2(p):
            p2 = st[p]["p2"]
            t3 = []
            for m in range(2):
                ps2 = pc2.tile([128, 392], F32)
                first = True
                for k in range(2):
                    pv = p2[k].rearrange("q (i r c) -> q i r c", i=2, r=29, c=32)
                    for tp in range(9):
                        ky, kx = divmod(tp, 3)
                        nc.tensor.matmul(
                            ps2[:],
                            w2t[:, tp, k, m * 128 : (m + 1) * 128],
                            pv[:, :, ky : min(ky + 28, 29) : 2, 1 + kx : 29 + kx : 2],
                            start=first,
                            stop=(k == 1 and tp == 8),
                        )
                        first = False
                t3m = t3p.tile([128, 392], BF16, tag=f"t3_{m}")
                t2 = rp.tile([128, 392], F32, tag="t1", name="t2")
                nc.scalar.activation(
                    t2[:], ps2[:], AFT.Relu, bias=beta2[:, m : m + 1], scale=a2
                )
                nc.vector.tensor_scalar(
                    t3m[:], t2[:], 127.25, 384.0, op0=ALU.min, op1=ALU.add
                )
                t3.append(t3m)
            st[p]["t3"] = t3

        def emit_conv3(p, last=False):
            xq, t3 = st[p]["xq"], st[p]["t3"]
            for m in range(8):
                ps3 = pc3.tile([128, 392], F32)
                # shortcut first: only needs xq, giving t3's epilogue time
                for k in range(4):
                    xv = xq[:, :, k].rearrange("q i (r c) -> q i r c", r=28, c=28)
                    nc.tensor.matmul(
                        ps3[:],
                        wst[:, k, m * 128 : (m + 1) * 128],
                        xv[:, :, 0:28:2, 0:28:2],
                        start=(k == 0),
                        stop=False,
                        skip_group_check=True,
                    )
                for k in range(2):
                    nc.tensor.matmul(
                        ps3[:],
                        w3t[:, k, m * 128 : (m + 1) * 128],
                        t3[k][:],
                        start=False,
                        stop=(k == 1),
                        skip_group_check=True,
                    )
                # single fused DVE epilogue + per-m store: keeps the chain
                # after each m's last matmul short, and keeps all elementwise
                # epilogue work OFF GpSimd (Q7 tensor ops starve SBUF
                # arbitration for every other engine)
                yo = yop.tile([128, 392], BF16, tag="yot", name="yot")
                nc.vector._custom_dve(
                    FEPI, out=yo[:], in0=ps3[:], in1=dlf[m][:],
                    s0=g3, s1=6.0,
                )
                nc.sync.dma_start(
                    y_d[p, m // 2][:, (m % 2) * 392 : (m % 2) * 392 + 392],
                    yo[:],
                )
            del st[p]

        # pair 0+1 x loads first (SP queue), then pair-0 conv1
        emit_xload(0, split_first=True, after_first=load_w1)
        emit_xload(1, fine=True)
        emit_conv1(0)

        # bulk weights: one consolidated DMA per tensor on the SP queue,
        # small biases via the GpSimd SWDGE queue
        w2t = wp.tile([128, 9, 2, 256], BF16, tag="w2t")
        nc.sync.dma_start(w2t[:], w2_d.rearrange("t k q n -> q t k n"))
        wst = wp.tile([128, 4, 1024], BF16, tag="wst")
        nc.sync.dma_start(wst[:], ws_d.rearrange("k q n -> q k n"))
        w3t = wp.tile([128, 2, 1024], BF16, tag="w3t")
        nc.sync.dma_start(w3t[:], w3_d.rearrange("k q n -> q k n"))
        beta2 = wp.tile([128, 2], F32, tag="beta2")
        nc.gpsimd.dma_start(beta2[:], b2_d[:])
        delta = wp.tile([128, 8], F32, tag="delta")
        nc.gpsimd.dma_start(delta[:], dl_d[:])
        dlf = []
        for j in range(8):
            # broadcast delta[:, j] along the free dim on ACT (idle during
            # the weight phase): Copy(0*rz + delta_j)
            t = wp.tile([128, 392], F32, tag=f"dlf{j}", name=f"dlf{j}")
            nc.scalar.activation(
                t[:], rz[:, :392], AFT.Identity, bias=delta[:, j : j + 1], scale=0.0
            )
            dlf.append(t)

        # software pipeline: conv2/conv3 of pair p-1 run under conv1 of pair p
        for p in range(1, pairs):
            emit_conv1(p)
            emit_conv2(p - 1)
            if p + 1 < pairs:
                emit_xload(p + 1)
            emit_conv3(p - 1)
        emit_conv2(pairs - 1)
        emit_conv3(pairs - 1, last=True)
    return nc


_SCALES = (1.0, 1.0, 1.0)


def _prep(w1, b1, w2, b2, w3, b3, wsw, bs):
    """Host-side weight quantization + constant folding (all tiny tensors)."""
    f32 = np.float32

    def qw(w):
        s = f32(np.max(np.abs(w)))
        wq = np.round(np.clip(w / s, f32(-1.0), f32(1.0)) * f32(127.0)).astype(
            np.float32
        )
        return wq, s

    def qb(b):
        return np.round(b * f32(127.0)).astype(np.float32)

    w1q, c1s = qw(w1)  # [256,512,1,1]
    w2q, c2s = qw(w2)  # [256,256,3,3]
    w3q, c3s = qw(w3)  # [1024,256,1,1]
    wsq, css = qw(wsw)  # [1024,512,1,1]
    B1, B2, B3, Bs = qb(b1), qb(b2), qb(b3), qb(bs)

    a1 = f32(2.0) * c1s / f32(127.0)
    a2 = f32(2.0) * c2s / f32(127.0)
    g3 = c3s / f32(2.0 * 16129.0)
    rho = css / c3s

    # lhsT layouts
    w1l = np.ascontiguousarray(
        w1q[:, :, 0, 0].T.reshape(4, 128, 256).astype(BF16NP)
    )
    # w2 taps: [ky,kx] -> lhsT [cin, cout] per tap
    w2l = np.ascontiguousarray(
        w2q.transpose(2, 3, 1, 0).reshape(9, 2, 128, 256).astype(BF16NP)
    )
    w3l = np.ascontiguousarray(
        w3q[:, :, 0, 0].T.reshape(2, 128, 1024).astype(BF16NP)
    )
    ws_sc = (rho * wsq[:, :, 0, 0]).astype(BF16NP)  # [1024,512] scaled bf16
    wsl = np.ascontiguousarray(ws_sc.T.reshape(4, 128, 1024))

    # column sums for the activation offset corrections (fp64 exact)
    K1 = w1q[:, :, 0, 0].astype(np.float64).sum(axis=1)  # [256]
    K2 = w2q.astype(np.float64).sum(axis=(1, 2, 3))  # [256]
    K3 = w3q[:, :, 0, 0].astype(np.float64).sum(axis=1)  # [1024]
    Ks = ws_sc.astype(np.float64).sum(axis=1)  # [1024]

    # activation shifts: x-quant adds +1151 (f16 trick), conv epis add +128
    beta1 = (
        f32(4.0) * B1
        - (a1.astype(np.float64) * 383.0 * K1).astype(np.float32)
    ).astype(np.float32)
    beta2 = (f32(4.0) * B2 - a2 * f32(384.0) * K2.astype(np.float32)).astype(
        np.float32
    )
    delta0 = B3 * c3s / (f32(127.0) * c2s) + Bs / f32(127.0)
    delta = (
        delta0
        - (g3.astype(np.float64) * (384.0 * K3 + 383.0 * Ks)).astype(np.float32)
    ).astype(np.float32)

    beta1 = np.ascontiguousarray(beta1.reshape(2, 128).T)  # [128,2]
    beta2 = np.ascontiguousarray(beta2.reshape(2, 128).T)
    delta = np.ascontiguousarray(delta.reshape(8, 128).T)  # [128,8]

    return dict(
        w1l=w1l, w2l=w2l, w3l=w3l, wsl=wsl,
        beta1=beta1, beta2=beta2, delta=delta,
        a1=float(a1), a2=float(a2), g3=float(g3),
    )


def kernel(x, w1, b1, w2, b2, w3, b3, ws, bs):
    global _SCALES
    x16 = np.asarray(x).astype(np.float16)
    pre = _prep(
        np.asarray(w1, np.float32), np.asarray(b1, np.float32),
        np.asarray(w2, np.float32), np.asarray(b2, np.float32),
        np.asarray(w3, np.float32), np.asarray(b3, np.float32),
        np.asarray(ws, np.float32), np.asarray(bs, np.float32),
    )
    _SCALES = (pre["a1"], pre["a2"], pre["g3"])
    nc = _build_nc()
    nc.compile()

    shared = {
        "w1l": pre["w1l"], "w2l": pre["w2l"], "w3l": pre["w3l"],
        "wsl": pre["wsl"], "beta1": pre["beta1"], "beta2": pre["beta2"],
        "delta": pre["delta"],
    }
    in_maps = []
    for c in range(N_CORES):
        xs = np.ascontiguousarray(
            x16[c * B_LOC : (c + 1) * B_LOC].reshape(B_LOC, 4, 128, 784)
        )
        in_maps.append({"x": xs, **shared})

    import os

    tmpdir = os.environ.get("KERNEL_TRACE_DIR") or None
    if tmpdir:
        os.makedirs(tmpdir, exist_ok=True)
    res = run_bass_kernel_spmd(nc, in_maps, list(range(N_CORES)), tmpdir=tmpdir)
    global LAST_RESULT
    LAST_RESULT = res
    outs = [unpack_y(res.results[c]["y"]) for c in range(N_CORES)]
    return np.ascontiguousarray(np.concatenate(outs, axis=0))


def unpack_y(y):
    """[pairs,4,128,784] packed -> [2*pairs, 1024, 14, 14]."""
    p = y.shape[0]
    y = y.reshape(p, 4, 128, 2, 2, 196)  # (p, mp, q, m2, i, h)
    y = y.transpose(0, 4, 1, 3, 2, 5)  # (p, i, mp, m2, q, h)
    return np.ascontiguousarray(
        y.reshape(2 * p, 1024, 14, 14).astype(np.float32)
    )

